# revision 24
# baseline (speedup 1.0000x reference)
"""Causal attention head kernel for Trainium2, 8 NeuronCores — v3.

Problem: B=4, S=4096, D_IN=512, D_OUT=64, f32, causal, scale=1/sqrt(S).

Sharding: core c -> (batch b = c//2, k-shard hk = c%2). Each core handles ALL
queries of its batch but only the k-tiles (of 128 rows) with tile_index % 2 ==
hk, producing partial (numerator | denominator) sums; the host combines the
two k-shards. SPMD: causality differences between the two k-shards live in a
per-core mask input and in the per-core gather of X_k/X_v rows.

v3 changes (vs the v2 fp16+f32r-corner baseline, 52573ns):
  * All-fp16 datapath: the f32r corner (q rows 0-511) is gone; chunk 0 is a
    normal diagonal-only chunk. Cuts ~2.9MB/core of duplicate f32 DMA.
  * exp split across TWO engines: the Activation engine runs true exp for
    chunk diagonals + early off-diag pairs; the DVE runs a Schraudolph
    bit-trick exp (int16(score*1024*log2e + (15-c)*1024) bitcast as fp16)
    for 15 late off-diag pairs (q >= 1536), alternating with Act inside each
    chunk so the per-pair exp latency halves. Numpy-validated rel RMS err
    ~7.8e-3 (gate 2e-2).
  * PSUM->SBUF traffic rebalanced: qt/kt/vaug copies on DVE, output staging
    copies (fp16) on the Activation engine, diagonal mask multiplies on the
    otherwise-idle GPSIMD (SBUF-only op; GPSIMD has no PSUM port). vproj
    does both 128-row halves in one PSUM accumulation group and one copy.
  * Outputs (numerator | denominator per chunk) staged to fp16 in SBUF,
    one [1024, 260] fp16 tensor (halves output DMA).
  * Inputs stream on the SP/HWDGE lane in compute-need order; masks on the
    gpsimd/SWDGE lane.
"""

import os

os.environ.setdefault("JAX_PLATFORMS", "cpu")

import numpy as np

import concourse.bass as bass
import concourse.bacc as bacc
import concourse.mybir as mybir
from concourse import tile
from concourse.bass_utils import run_bass_kernel_spmd

F32 = mybir.dt.float32
F16 = mybir.dt.float16
I16 = mybir.dt.int16

B, S, D_IN, D_OUT = 4, 4096, 512, 64
SK = S // 2          # per-core k rows (interleaved 128-tiles)
N_KT = SK // 128     # 16 local k-tiles
N_QC = S // 512      # 8 q-chunks of 512
N_CORES = 8
EXP = mybir.ActivationFunctionType.Exp

# Schraudolph fp16 exp: u = int16(y*1024*log2(e) + (15 - c)*1024), bitcast
# fp16 gives exp(y)*(1 + ~1.9% rms ripple). c centers the ripple.
SCH_A = float(1024.0 * np.log2(np.e))
SCH_C = 0.055
SCH_B = float((15.0 - SCH_C) * 1024.0)

# Off-diagonal (chunk, pair) exp assignments for the DVE (late chunks only:
# their rows average over >=1536 keys so the ripple washes out). Chosen to
# alternate Act/DVE inside each chunk (diag exp is always Act).
DVE_PAIRS = {(3, 1), (4, 1), (4, 3), (5, 0), (5, 2), (5, 4),
             (6, 1), (6, 3), (6, 5),
             (7, 0), (7, 2), (7, 4), (7, 6)}

N_WARMUP = 20

_CACHE = {}


def mm(nc, out, lhsT, rhs, start, stop):
    nc.tensor.matmul(out, lhsT, rhs, start=start, stop=stop)


def build_nc():
    nc = bacc.Bacc(trn_type="TRN2", target_bir_lowering=False, debug=False)

    xqh = nc.dram_tensor("xqh", [128, 4, S], F16, kind="ExternalInput").ap()
    xkvh = nc.dram_tensor("xkvh", [128, 8, SK], F16, kind="ExternalInput").ap()
    w_h = nc.dram_tensor("w_h", [128, 12, D_OUT], F16, kind="ExternalInput").ap()
    cm_h = nc.dram_tensor("cm_h", [128, 768], F16, kind="ExternalInput").ap()
    # output: partial (numerator | denominator); chunk j in rows 128j..128j+128
    pvtl = nc.dram_tensor("pvtl", [N_QC * 128, 260], F16, kind="ExternalOutput").ap()

    with tile.TileContext(nc) as tc:
        with (
            tc.tile_pool(name="persist", bufs=1) as pp,
            tc.tile_pool(name="et", bufs=6) as etp,
            tc.tile_pool(name="ostage", bufs=4) as osp,
            # one shared 3-deep [128,1024] PSUM pool for scores, projections
            # and warmup (6 banks) + 2-deep [128,512] for PV accumulation
            # (2 banks). 3-deep score tiles let s_pair(i+2) start while
            # exp(i) is still in flight.
            tc.tile_pool(name="ps_s", bufs=3, space="PSUM") as ps_s,
            tc.tile_pool(name="ps_pv", bufs=2, space="PSUM") as ps_pv,
        ):
            # ---- persistent SBUF tiles ----
            xqT = pp.tile([128, 4, S], F16, tag="xqT", name="xqT")
            xkvT = pp.tile([128, 8, SK], F16, tag="xkvT", name="xkvT")
            qt = pp.tile([64, S], F16, tag="qt", name="qt")
            kt = pp.tile([64, SK], F16, tag="kt", name="kt")
            vaug = pp.tile([128, N_KT, D_OUT + 2], F16, tag="vaug", name="vaug")
            cmask_h = pp.tile([128, 768], F16, tag="cmask_h", name="cmask_h")
            wsb_h = pp.tile([128, 12, D_OUT], F16, tag="wsb_h", name="wsb_h")
            wq_h, wk_h, wv_h = (wsb_h[:, 0:4, :], wsb_h[:, 4:8, :],
                                wsb_h[:, 8:12, :])

            # PE warmup: dummy matmuls with no DMA dependency keep the PE
            # busy (and its p-state ramping) while the first loads land.
            wu = pp.tile([128, 320], F16, tag="wu", name="wu")
            nc.gpsimd.memset(wu[:].opt().bitcast(mybir.dt.uint32), 0)
            for n in range(N_WARMUP):
                ps = ps_s.tile([128, 1024], F32, tag="ps_s", name=f"wu{n}")
                mm(nc, ps[0:64, 0:256], wu[:, 0:64], wu[:, 64:320],
                   start=True, stop=True)

            # ones columns for the softmax denominator (V copies overwrite
            # cols 0:64 of each slot; col 64 stays at the memset value)
            nc.vector.memset(vaug[:].opt().bitcast(mybir.dt.uint32),
                             0x3C003C00)  # fp16 1.0 pair

            def load_q(lane, c0, c1):
                lane.dma_start(out=xqT[:, :, c0:c1], in_=xqh[:, :, c0:c1])

            def load_kv(lane, p):
                sl = slice(p * 256, (p + 1) * 256)
                lane.dma_start(out=xkvT[:, :, sl], in_=xkvh[:, :, sl])

            # Loads in compute-need order. SP issues through HWDGE, gpsimd
            # through SWDGE - the two DGE paths run concurrently.
            sp, gp = nc.sync, nc.gpsimd
            gp.dma_start(out=cmask_h[:], in_=cm_h[:])
            sp.dma_start(out=wsb_h[:], in_=w_h[:])
            # chunk 0 needs: kv pair 0 (k part first) + q cols 0:512;
            # q loads follow the chunk processing order (chunk 1 is last)
            sp.dma_start(out=xkvT[:, 0:4, 0:256], in_=xkvh[:, 0:4, 0:256])
            load_q(sp, 0, 512)
            sp.dma_start(out=xkvT[:, 4:8, 0:256], in_=xkvh[:, 4:8, 0:256])
            for j in range(1, N_QC):
                load_kv(sp, j)
                load_q(sp, 512 * j, 512 * (j + 1))

            def kproj(p):
                # k-pair p: tiles 2p, 2p+1 (cols 256p : 256p+256)
                sl = slice(p * 256, (p + 1) * 256)
                ps = ps_s.tile([128, 1024], F32, tag="ps_s", name=f"pp_k{p}")
                for dt in range(4):
                    mm(nc, ps[0:64, 0:256], wk_h[:, dt, :],
                       xkvT[:, dt, sl], start=(dt == 0), stop=(dt == 3))
                nc.vector.tensor_copy(kt[:, sl], ps[0:64, 0:256])

            def vproj(p):
                # both 128-row halves in one accumulation group, one copy
                ps = ps_s.tile([128, 1024], F32, tag="ps_s", name=f"pp_v{p}")
                nmm = 0
                for h in range(2):
                    l = 2 * p + h
                    for dt in range(4):
                        mm(nc, ps[:, 64 * h:64 * h + D_OUT],
                           xkvT[:, 4 + dt, l * 128:(l + 1) * 128],
                           wv_h[:, dt, :],
                           start=(nmm == 0), stop=(nmm == 7))
                        nmm += 1
                src = ps.rearrange("p (h e) -> p h e", h=16)
                nc.vector.tensor_copy(vaug[:, 2 * p:2 * p + 2, 0:D_OUT],
                                      src[:, 0:2, :])

            def q_stage(j):
                sl = slice(j * 512, (j + 1) * 512)
                ps = ps_s.tile([128, 1024], F32, tag="ps_s", name=f"pp_q{j}")
                for dt in range(4):
                    mm(nc, ps[0:64, 0:512], wq_h[:, dt, :], xqT[:, dt, sl],
                       start=(dt == 0), stop=(dt == 3))
                nc.vector.tensor_copy(qt[:, sl], ps[0:64, 0:512])

            def emit_out(j, pv):
                # stage (numerator|denominator) to fp16 SBUF on the
                # Activation engine, then DMA. Called from the NEXT chunk so
                # its wait is satisfied by the time it reaches the Act queue.
                ost = osp.tile([128, 260], F16, tag="ost", name=f"ost{j}")
                src = pv.rearrange("p (t c) -> p t c", t=4)
                dst = ost.rearrange("p (t c) -> p t c", t=4, c=65)
                nc.scalar.copy(dst[:], src[:, :, 0:65])
                lane = nc.sync if j == N_QC - 1 else nc.gpsimd
                lane.dma_start(out=pvtl[j * 128:(j + 1) * 128, :], in_=ost[:])

            def chunk(j, hooks=None, diag_first=False, split_exp=False):
                qs = qt[:, j * 512:(j + 1) * 512]
                pv = ps_pv.tile([128, 512], F32, tag="ps_pv", name=f"pv{j}")
                ets = {}
                # diag at position 2: early enough that its exp->mask->PV
                # chain overlaps later pairs (not the chunk tail), late
                # enough that it doesn't serialize the chunk start (pair 0
                # only needs long-loaded kt columns).
                if diag_first:
                    order = [j] + list(range(j))
                elif j <= 1:
                    order = list(range(j)) + [j]
                else:
                    order = [0, 1, j] + list(range(2, j))

                def s_pair(i):
                    ps = ps_s.tile([128, 1024], F32, tag="ps_s",
                                   name=f"st{j}_{i}")
                    mm(nc, ps[:, 0:512], kt[:, (2 * i) * 128:(2 * i + 1) * 128],
                       qs, start=True, stop=True)
                    if i < j:
                        mm(nc, ps[:, 512:1024],
                           kt[:, (2 * i + 1) * 128:(2 * i + 2) * 128], qs,
                           start=True, stop=True)
                        et = etp.tile([128, 1024], F16, tag="et",
                                      name=f"et{j}_{i}")
                        if split_exp and i == order[-1]:
                            # tail-latency trim: halves on Act and DVE in
                            # parallel
                            nc.scalar.activation(et[:, 0:512], ps[:, 0:512],
                                                 EXP)
                            nc.vector.tensor_scalar(
                                et[:, 512:1024].bitcast(I16), ps[:, 512:1024],
                                SCH_A, SCH_B,
                                mybir.AluOpType.mult, mybir.AluOpType.add)
                        elif (j, i) in DVE_PAIRS:
                            nc.vector.tensor_scalar(
                                et[:].bitcast(I16), ps[:], SCH_A, SCH_B,
                                mybir.AluOpType.mult, mybir.AluOpType.add)
                        else:
                            nc.scalar.activation(et[:], ps[:], EXP)
                    else:
                        mm(nc, ps[:, 512:768],
                           kt[:, (2 * i + 1) * 128:(2 * i + 2) * 128],
                           qs[:, 256:512], start=True, stop=True)
                        et = etp.tile([128, 1024], F16, tag="et",
                                      name=f"et{j}_{i}")
                        nc.scalar.activation(et[:, 0:768], ps[:, 0:768], EXP)
                        # mask split across DVE (h0) and Pool (h1) so the
                        # exp->mask->PV chain is ~650ns, not one 1.6us op
                        nc.vector.tensor_mul(
                            et[:, 0:512], et[:, 0:512], cmask_h[:, 0:512])
                        nc.gpsimd.tensor_mul(
                            et[:, 512:768], et[:, 512:768],
                            cmask_h[:, 512:768])
                    ets[i] = et

                def pv_pair(i, first, last):
                    et = ets[i]
                    mms = []
                    for t in range(4):
                        for h in range(2):
                            if i == j and h == 1 and t < 2:
                                continue
                            l = 2 * i + h
                            if h == 0:
                                esl = slice(128 * t, 128 * t + 128)
                            elif i < j:
                                esl = slice(512 + 128 * t, 512 + 128 * t + 128)
                            else:
                                esl = slice(512 + 128 * (t - 2),
                                            512 + 128 * (t - 2) + 128)
                            mms.append((t, et[:, esl], vaug[:, l, 0:D_OUT + 1]))
                    for n, (t, lh, rh) in enumerate(mms):
                        mm(nc, pv[:, t * 128:t * 128 + 65], lh, rh,
                           start=(first and n == 0),
                           stop=(last and n == len(mms) - 1))

                for n, i in enumerate(order):
                    s_pair(i)
                    for f in (hooks or {}).get(n, ()):
                        f()
                    if n > 0:
                        pv_pair(order[n - 1], first=(n == 1), last=False)
                pv_pair(order[-1], first=(j == 0), last=True)
                return pv

            import functools
            corder = list(range(N_QC))
            q_stage(0)
            kproj(0)
            prev = None
            kv_done = 1   # kproj/vproj emitted for pairs < kv_done (kproj 0 above)
            for n, j in enumerate(corder):
                nxt = corder[n + 1] if n + 1 < len(corder) else None
                hooks = {0: [], 1: []}
                hooks[0] += [functools.partial(vproj, 0)] if j == 0 else []
                while kv_done <= j:
                    hooks[0] += [functools.partial(kproj, kv_done),
                                 functools.partial(vproj, kv_done)]
                    kv_done += 1
                if nxt is not None:
                    hooks[0] += [functools.partial(q_stage, nxt)]
                if prev is not None:
                    hooks[min(1, j)] += [functools.partial(emit_out, *prev)]
                prev = (j, chunk(j, hooks=hooks,
                                 split_exp=(j == corder[-1])))
            emit_out(*prev)
    nc.compile()
    return nc


def _prep_w(w, scale=1.0):
    # [512, 64] -> [128, 4, 64]: (p, dt, e) holds W[dt*128 + p, e]
    return np.ascontiguousarray(
        (w * scale).reshape(4, 128, D_OUT).transpose(1, 0, 2).astype(np.float32))


def _dblock(xT):
    # [512, C] -> [128, 4, C]
    return np.ascontiguousarray(
        xT.reshape(4, 128, -1).transpose(1, 0, 2))


def kernel(inputs_for_keys, inputs_for_values, inputs_for_queries, WK, WV, WQ):
    xk_f = np.asarray(inputs_for_keys, np.float32)
    xv_f = np.asarray(inputs_for_values, np.float32)
    xq_f = np.asarray(inputs_for_queries, np.float32)
    wkp = _prep_w(np.asarray(WK, np.float32))
    wvp = _prep_w(np.asarray(WV, np.float32))
    wqp = _prep_w(np.asarray(WQ, np.float32), scale=1.0 / np.sqrt(np.float32(S)))
    wcat = np.concatenate([wqp, wkp, wvp], axis=1)  # [128, 12, 64]

    if "nc" not in _CACHE:
        _CACHE["nc"] = build_nc()
    nc = _CACHE["nc"]

    # cmask[p, c]       (c in 0:512):  1 if c >= p + 128*hk      (diag h=0)
    # cmask[p, 512+cc]  (cc in 0:256): 1 if cc >= p + 128*hk     (diag h=1)
    kk = np.arange(128)[:, None]
    cc512 = np.arange(512)[None, :]
    cms = []
    for hk in range(2):
        m0 = (cc512 >= kk + 128 * hk).astype(np.float32)
        cms.append(np.ascontiguousarray(
            np.concatenate([m0, m0[:, 0:256]], axis=1)))

    in_maps = []
    for c in range(N_CORES):
        b, hk = c // 2, c % 2
        xk_g = xk_f[b].reshape(S // 128, 128, D_IN)[hk::2].reshape(SK, D_IN)
        xv_g = xv_f[b].reshape(S // 128, 128, D_IN)[hk::2].reshape(SK, D_IN)
        xq_db = _dblock(xq_f[b].T)
        xkv_db = np.concatenate([_dblock(xk_g.T), _dblock(xv_g.T)], axis=1)
        in_maps.append({
            "xqh": xq_db.astype(np.float16),
            "xkvh": xkv_db.astype(np.float16),
            "w_h": wcat.astype(np.float16),
            "cm_h": cms[hk].astype(np.float16),
        })

    _CACHE["in_maps"] = in_maps
    res = run_bass_kernel_spmd(nc, in_maps, core_ids=list(range(N_CORES)))
    out = np.empty((B, S, D_OUT), np.float32)
    for b in range(B):
        full = np.empty((S, D_OUT + 1), np.float32)
        for kshard in range(2):
            r = res.results[2 * b + kshard]
            part = (r["pvtl"].astype(np.float32).reshape(N_QC, 128, 4, 65)
                    .transpose(0, 2, 1, 3).reshape(S, 65))
            if kshard == 0:
                full[:] = part
            else:
                full += part
        out[b] = full[:, 0:D_OUT] / full[:, D_OUT:D_OUT + 1]
    return out


# revision 25
# speedup vs baseline: 1.0290x; 1.0290x over previous
"""Causal attention head kernel for Trainium2, 8 NeuronCores — v3.

Problem: B=4, S=4096, D_IN=512, D_OUT=64, f32, causal, scale=1/sqrt(S).

Sharding: core c -> (batch b = c//2, k-shard hk = c%2). Each core handles ALL
queries of its batch but only the k-tiles (of 128 rows) with tile_index % 2 ==
hk, producing partial (numerator | denominator) sums; the host combines the
two k-shards. SPMD: causality differences between the two k-shards live in a
per-core mask input and in the per-core gather of X_k/X_v rows.

v3 changes (vs the v2 fp16+f32r-corner baseline, 52573ns):
  * All-fp16 datapath: the f32r corner (q rows 0-511) is gone; chunk 0 is a
    normal diagonal-only chunk. Cuts ~2.9MB/core of duplicate f32 DMA.
  * exp split across TWO engines: the Activation engine runs true exp for
    chunk diagonals + early off-diag pairs; the DVE runs a Schraudolph
    bit-trick exp (int16(score*1024*log2e + (15-c)*1024) bitcast as fp16)
    for 15 late off-diag pairs (q >= 1536), alternating with Act inside each
    chunk so the per-pair exp latency halves. Numpy-validated rel RMS err
    ~7.8e-3 (gate 2e-2).
  * PSUM->SBUF traffic rebalanced: qt/kt/vaug copies on DVE, output staging
    copies (fp16) on the Activation engine, diagonal mask multiplies on the
    otherwise-idle GPSIMD (SBUF-only op; GPSIMD has no PSUM port). vproj
    does both 128-row halves in one PSUM accumulation group and one copy.
  * Outputs (numerator | denominator per chunk) staged to fp16 in SBUF,
    one [1024, 260] fp16 tensor (halves output DMA).
  * Inputs stream on the SP/HWDGE lane in compute-need order; masks on the
    gpsimd/SWDGE lane.
"""

import os

os.environ.setdefault("JAX_PLATFORMS", "cpu")

import numpy as np

import concourse.bass as bass
import concourse.bacc as bacc
import concourse.mybir as mybir
from concourse import tile
from concourse.bass_utils import run_bass_kernel_spmd

F32 = mybir.dt.float32
F16 = mybir.dt.float16
I16 = mybir.dt.int16

B, S, D_IN, D_OUT = 4, 4096, 512, 64
SK = S // 2          # per-core k rows (interleaved 128-tiles)
N_KT = SK // 128     # 16 local k-tiles
N_QC = S // 512      # 8 q-chunks of 512
N_CORES = 8
EXP = mybir.ActivationFunctionType.Exp

# Schraudolph fp16 exp: u = int16(y*1024*log2(e) + (15 - c)*1024), bitcast
# fp16 gives exp(y)*(1 + ~1.9% rms ripple). c centers the ripple.
SCH_A = float(1024.0 * np.log2(np.e))
SCH_C = 0.055
SCH_B = float((15.0 - SCH_C) * 1024.0)

# Off-diagonal (chunk, pair) exp assignments for the DVE (late chunks only:
# their rows average over >=1536 keys so the ripple washes out). Chosen to
# alternate Act/DVE inside each chunk (diag exp is always Act).
DVE_PAIRS = {(3, 1), (4, 1), (4, 3), (5, 0), (5, 2), (5, 4),
             (6, 1), (6, 3), (6, 5),
             (7, 0), (7, 2), (7, 4), (7, 6)}

N_WARMUP = 20

_CACHE = {}


def mm(nc, out, lhsT, rhs, start, stop):
    nc.tensor.matmul(out, lhsT, rhs, start=start, stop=stop)


def build_nc():
    nc = bacc.Bacc(trn_type="TRN2", target_bir_lowering=False, debug=False)

    xqh = nc.dram_tensor("xqh", [128, 4, S], F16, kind="ExternalInput").ap()
    xkvh = nc.dram_tensor("xkvh", [128, 8, SK], F16, kind="ExternalInput").ap()
    w_h = nc.dram_tensor("w_h", [128, 12, D_OUT], F16, kind="ExternalInput").ap()
    cm_h = nc.dram_tensor("cm_h", [128, 768], F16, kind="ExternalInput").ap()
    # output: partial (numerator | denominator); chunk j in rows 128j..128j+128
    pvtl = nc.dram_tensor("pvtl", [N_QC * 128, 260], F16, kind="ExternalOutput").ap()

    with tile.TileContext(nc) as tc:
        with (
            tc.tile_pool(name="persist", bufs=1) as pp,
            tc.tile_pool(name="et", bufs=6) as etp,
            tc.tile_pool(name="ostage", bufs=4) as osp,
            # one shared 3-deep [128,1024] PSUM pool for scores, projections
            # and warmup (6 banks) + 2-deep [128,512] for PV accumulation
            # (2 banks). 3-deep score tiles let s_pair(i+2) start while
            # exp(i) is still in flight.
            tc.tile_pool(name="ps_s", bufs=3, space="PSUM") as ps_s,
            tc.tile_pool(name="ps_pv", bufs=2, space="PSUM") as ps_pv,
        ):
            # ---- persistent SBUF tiles ----
            xqT = pp.tile([128, 4, S], F16, tag="xqT", name="xqT")
            xkvT = pp.tile([128, 8, SK], F16, tag="xkvT", name="xkvT")
            qt = pp.tile([64, S], F16, tag="qt", name="qt")
            kt = pp.tile([64, SK], F16, tag="kt", name="kt")
            vaug = pp.tile([128, N_KT, D_OUT + 2], F16, tag="vaug", name="vaug")
            cmask_h = pp.tile([128, 768], F16, tag="cmask_h", name="cmask_h")
            wsb_h = pp.tile([128, 12, D_OUT], F16, tag="wsb_h", name="wsb_h")
            wq_h, wk_h, wv_h = (wsb_h[:, 0:4, :], wsb_h[:, 4:8, :],
                                wsb_h[:, 8:12, :])

            # PE warmup: dummy matmuls with no DMA dependency keep the PE
            # busy (and its p-state ramping) while the first loads land.
            wu = pp.tile([128, 320], F16, tag="wu", name="wu")
            nc.gpsimd.memset(wu[:].opt().bitcast(mybir.dt.uint32), 0)
            for n in range(N_WARMUP):
                ps = ps_s.tile([128, 1024], F32, tag="ps_s", name=f"wu{n}")
                mm(nc, ps[0:64, 0:256], wu[:, 0:64], wu[:, 64:320],
                   start=True, stop=True)

            # ones columns for the softmax denominator (V copies overwrite
            # cols 0:64 of each slot; col 64 stays at the memset value)
            nc.vector.memset(vaug[:].opt().bitcast(mybir.dt.uint32),
                             0x3C003C00)  # fp16 1.0 pair

            def load_q(lane, c0, c1):
                lane.dma_start(out=xqT[:, :, c0:c1], in_=xqh[:, :, c0:c1])

            def load_kv(lane, p):
                sl = slice(p * 256, (p + 1) * 256)
                lane.dma_start(out=xkvT[:, :, sl], in_=xkvh[:, :, sl])

            # Loads in compute-need order. SP issues through HWDGE, gpsimd
            # through SWDGE - the two DGE paths run concurrently.
            sp, gp = nc.sync, nc.gpsimd
            gp.dma_start(out=cmask_h[:], in_=cm_h[:])
            sp.dma_start(out=wsb_h[:], in_=w_h[:])
            # chunk 0 needs: kv pair 0 (k part first) + q cols 0:512;
            # q loads follow the chunk processing order (chunk 1 is last)
            sp.dma_start(out=xkvT[:, 0:4, 0:256], in_=xkvh[:, 0:4, 0:256])
            load_q(sp, 0, 512)
            sp.dma_start(out=xkvT[:, 4:8, 0:256], in_=xkvh[:, 4:8, 0:256])
            for j in range(1, N_QC):
                load_kv(sp, j)
                load_q(sp, 512 * j, 512 * (j + 1))

            def kproj(p):
                # k-pair p: tiles 2p, 2p+1 (cols 256p : 256p+256)
                sl = slice(p * 256, (p + 1) * 256)
                ps = ps_s.tile([128, 1024], F32, tag="ps_s", name=f"pp_k{p}")
                for dt in range(4):
                    mm(nc, ps[0:64, 0:256], wk_h[:, dt, :],
                       xkvT[:, dt, sl], start=(dt == 0), stop=(dt == 3))
                nc.vector.tensor_copy(kt[:, sl], ps[0:64, 0:256])

            def vproj(p):
                # both 128-row halves in one accumulation group, one copy
                ps = ps_s.tile([128, 1024], F32, tag="ps_s", name=f"pp_v{p}")
                nmm = 0
                for h in range(2):
                    l = 2 * p + h
                    for dt in range(4):
                        mm(nc, ps[:, 64 * h:64 * h + D_OUT],
                           xkvT[:, 4 + dt, l * 128:(l + 1) * 128],
                           wv_h[:, dt, :],
                           start=(nmm == 0), stop=(nmm == 7))
                        nmm += 1
                src = ps.rearrange("p (h e) -> p h e", h=16)
                nc.vector.tensor_copy(vaug[:, 2 * p:2 * p + 2, 0:D_OUT],
                                      src[:, 0:2, :])

            def q_stage(j):
                sl = slice(j * 512, (j + 1) * 512)
                ps = ps_s.tile([128, 1024], F32, tag="ps_s", name=f"pp_q{j}")
                for dt in range(4):
                    mm(nc, ps[0:64, 0:512], wq_h[:, dt, :], xqT[:, dt, sl],
                       start=(dt == 0), stop=(dt == 3))
                nc.vector.tensor_copy(qt[:, sl], ps[0:64, 0:512])

            def emit_out(j, pv):
                # stage (numerator|denominator) to fp16 SBUF on the
                # Activation engine, then DMA. Called from the NEXT chunk so
                # its wait is satisfied by the time it reaches the Act queue.
                ost = osp.tile([128, 260], F16, tag="ost", name=f"ost{j}")
                src = pv.rearrange("p (t c) -> p t c", t=4)
                dst = ost.rearrange("p (t c) -> p t c", t=4, c=65)
                nc.scalar.copy(dst[:], src[:, :, 0:65])
                lane = nc.sync if j == N_QC - 1 else nc.gpsimd
                lane.dma_start(out=pvtl[j * 128:(j + 1) * 128, :], in_=ost[:])

            def chunk(j, hooks=None, diag_first=False, split_exp=False):
                qs = qt[:, j * 512:(j + 1) * 512]
                pv = ps_pv.tile([128, 512], F32, tag="ps_pv", name=f"pv{j}")
                ets = {}
                # diag at position 2: early enough that its exp->mask->PV
                # chain overlaps later pairs (not the chunk tail), late
                # enough that it doesn't serialize the chunk start (pair 0
                # only needs long-loaded kt columns).
                if diag_first:
                    order = [j] + list(range(j))
                elif j <= 1:
                    order = list(range(j)) + [j]
                else:
                    order = [0, 1, j] + list(range(2, j))

                def s_pair(i):
                    ps = ps_s.tile([128, 1024], F32, tag="ps_s",
                                   name=f"st{j}_{i}")
                    mm(nc, ps[:, 0:512], kt[:, (2 * i) * 128:(2 * i + 1) * 128],
                       qs, start=True, stop=True)
                    if i < j:
                        mm(nc, ps[:, 512:1024],
                           kt[:, (2 * i + 1) * 128:(2 * i + 2) * 128], qs,
                           start=True, stop=True)
                        et = etp.tile([128, 1024], F16, tag="et",
                                      name=f"et{j}_{i}")
                        if split_exp and i == order[-1]:
                            # tail-latency trim: halves on Act and DVE in
                            # parallel
                            nc.scalar.activation(et[:, 0:512], ps[:, 0:512],
                                                 EXP)
                            nc.vector.tensor_scalar(
                                et[:, 512:1024].bitcast(I16), ps[:, 512:1024],
                                SCH_A, SCH_B,
                                mybir.AluOpType.mult, mybir.AluOpType.add)
                        elif (j, i) in DVE_PAIRS:
                            nc.vector.tensor_scalar(
                                et[:].bitcast(I16), ps[:], SCH_A, SCH_B,
                                mybir.AluOpType.mult, mybir.AluOpType.add)
                        else:
                            nc.scalar.activation(et[:], ps[:], EXP)
                    else:
                        mm(nc, ps[:, 512:768],
                           kt[:, (2 * i + 1) * 128:(2 * i + 2) * 128],
                           qs[:, 256:512], start=True, stop=True)
                        et = etp.tile([128, 1024], F16, tag="et",
                                      name=f"et{j}_{i}")
                        nc.scalar.activation(et[:, 0:768], ps[:, 0:768], EXP)
                        # mask split across DVE (h0) and Pool (h1) so the
                        # exp->mask->PV chain is ~650ns, not one 1.6us op
                        nc.vector.tensor_mul(
                            et[:, 0:512], et[:, 0:512], cmask_h[:, 0:512])
                        nc.gpsimd.tensor_mul(
                            et[:, 512:768], et[:, 512:768],
                            cmask_h[:, 512:768])
                    ets[i] = et

                def pv_pair(i, first, last):
                    et = ets[i]
                    mms = []
                    for t in range(4):
                        for h in range(2):
                            if i == j and h == 1 and t < 2:
                                continue
                            l = 2 * i + h
                            if h == 0:
                                esl = slice(128 * t, 128 * t + 128)
                            elif i < j:
                                esl = slice(512 + 128 * t, 512 + 128 * t + 128)
                            else:
                                esl = slice(512 + 128 * (t - 2),
                                            512 + 128 * (t - 2) + 128)
                            mms.append((t, et[:, esl], vaug[:, l, 0:D_OUT + 1]))
                    for n, (t, lh, rh) in enumerate(mms):
                        mm(nc, pv[:, t * 128:t * 128 + 65], lh, rh,
                           start=(first and n == 0),
                           stop=(last and n == len(mms) - 1))

                for n, i in enumerate(order):
                    s_pair(i)
                    for f in (hooks or {}).get(n, ()):
                        f()
                    if n > 0:
                        pv_pair(order[n - 1], first=(n == 1), last=False)
                pv_pair(order[-1], first=(j == 0), last=True)
                return pv

            import functools
            corder = list(range(N_QC))
            q_stage(0)
            kproj(0)
            prev = None
            kv_done = 1   # kproj/vproj emitted for pairs < kv_done (kproj 0 above)
            for n, j in enumerate(corder):
                nxt = corder[n + 1] if n + 1 < len(corder) else None
                hooks = {0: [], 1: []}
                hooks[0] += [functools.partial(vproj, 0)] if j == 0 else []
                while kv_done <= j:
                    hooks[0] += [functools.partial(kproj, kv_done),
                                 functools.partial(vproj, kv_done)]
                    kv_done += 1
                if nxt is not None:
                    hooks[1 if j > 0 else 0] += [functools.partial(q_stage, nxt)]
                if prev is not None:
                    hooks[min(1, j)] += [functools.partial(emit_out, *prev)]
                prev = (j, chunk(j, hooks=hooks,
                                 split_exp=(j == corder[-1])))
            emit_out(*prev)
    nc.compile()
    return nc


def _prep_w(w, scale=1.0):
    # [512, 64] -> [128, 4, 64]: (p, dt, e) holds W[dt*128 + p, e]
    return np.ascontiguousarray(
        (w * scale).reshape(4, 128, D_OUT).transpose(1, 0, 2).astype(np.float32))


def _dblock(xT):
    # [512, C] -> [128, 4, C]
    return np.ascontiguousarray(
        xT.reshape(4, 128, -1).transpose(1, 0, 2))


def kernel(inputs_for_keys, inputs_for_values, inputs_for_queries, WK, WV, WQ):
    xk_f = np.asarray(inputs_for_keys, np.float32)
    xv_f = np.asarray(inputs_for_values, np.float32)
    xq_f = np.asarray(inputs_for_queries, np.float32)
    wkp = _prep_w(np.asarray(WK, np.float32))
    wvp = _prep_w(np.asarray(WV, np.float32))
    wqp = _prep_w(np.asarray(WQ, np.float32), scale=1.0 / np.sqrt(np.float32(S)))
    wcat = np.concatenate([wqp, wkp, wvp], axis=1)  # [128, 12, 64]

    if "nc" not in _CACHE:
        _CACHE["nc"] = build_nc()
    nc = _CACHE["nc"]

    # cmask[p, c]       (c in 0:512):  1 if c >= p + 128*hk      (diag h=0)
    # cmask[p, 512+cc]  (cc in 0:256): 1 if cc >= p + 128*hk     (diag h=1)
    kk = np.arange(128)[:, None]
    cc512 = np.arange(512)[None, :]
    cms = []
    for hk in range(2):
        m0 = (cc512 >= kk + 128 * hk).astype(np.float32)
        cms.append(np.ascontiguousarray(
            np.concatenate([m0, m0[:, 0:256]], axis=1)))

    in_maps = []
    for c in range(N_CORES):
        b, hk = c // 2, c % 2
        xk_g = xk_f[b].reshape(S // 128, 128, D_IN)[hk::2].reshape(SK, D_IN)
        xv_g = xv_f[b].reshape(S // 128, 128, D_IN)[hk::2].reshape(SK, D_IN)
        xq_db = _dblock(xq_f[b].T)
        xkv_db = np.concatenate([_dblock(xk_g.T), _dblock(xv_g.T)], axis=1)
        in_maps.append({
            "xqh": xq_db.astype(np.float16),
            "xkvh": xkv_db.astype(np.float16),
            "w_h": wcat.astype(np.float16),
            "cm_h": cms[hk].astype(np.float16),
        })

    _CACHE["in_maps"] = in_maps
    res = run_bass_kernel_spmd(nc, in_maps, core_ids=list(range(N_CORES)))
    out = np.empty((B, S, D_OUT), np.float32)
    for b in range(B):
        full = np.empty((S, D_OUT + 1), np.float32)
        for kshard in range(2):
            r = res.results[2 * b + kshard]
            part = (r["pvtl"].astype(np.float32).reshape(N_QC, 128, 4, 65)
                    .transpose(0, 2, 1, 3).reshape(S, 65))
            if kshard == 0:
                full[:] = part
            else:
                full += part
        out[b] = full[:, 0:D_OUT] / full[:, D_OUT:D_OUT + 1]
    return out
